# revision 94
# baseline (speedup 1.0000x reference)
"""CapsuleLayer (dynamic routing) Trainium2 kernel.

Self-contained: shards the full inputs over 8 NeuronCores (data-parallel over
batch), runs a Bass/Tile kernel per core, gathers the full output.

Shapes (full): u [256, 1152, 8] f32, W [1152, 10, 16, 8] f32 -> v [256, 10, 16].
Per core: B=32 batches, W replicated.

Math (per core, ROUTING_ITERS=3):
  u_hat[b,i,od] = sum_k W[i,od,k] * u[b,i,k]          (od = o*16+d)
  b0 = 0; for t in 0..2: c = softmax(b, o); s = sum_i c*u_hat; v = squash(s);
  if t<2: b += sum_d u_hat*v

t=0 shortcut: c uniform (0.1) -> s0 = 0.1*sum_i u_hat computed densely from
(ut, wr) without u_hat; v0 = squash(s0). The per-block t=0 recompute that the
first version carried was dead work and is removed.

All matmul operands are bf16 (PE: 1 cyc/row vs 4 for fp32); psum stays fp32.

Device layouts (i = jj*16+g, jj<72, g<16; partitions in [.]):
  Wr  [(g,k)=128, (jj,od)=11520] bf16  (host-pretransposed W)
  uT  [(g,k)=128, (jj,b)=2304]   bf16  (host-pretransposed u shard)
  BDu [(g,k)=128, (jj,b8,g')]    bf16  block-diag u, host-packed, DMA-streamed
  u_hat [(b8,g16)=128, (jj,od)]  bf16  built by PE: BDu.T @ Wr (per 8-batch blk)
  s matmul: lhsT = block-diag c bf16, rhs = u_hat -> psum[(b',o),od] f32
"""

import os
import sys

import numpy as np

for _p in ("/opt/trn_rl_repo", "/root/.axon_site/_ro/trn_rl_repo"):
    if os.path.isdir(_p) and _p not in sys.path:
        sys.path.insert(0, _p)

import concourse.bacc as bacc
import concourse.bass as bass
import concourse.mybir as mybir
import concourse.tile as tile

F32 = mybir.dt.float32
BF16 = mybir.dt.bfloat16


def _register_scan_mac():
    """Custom DVE op: out[p,k] = cumsum_k(in0*in1) (fp32 state).

    Used for the agreement step: running sum of u_hat*v, with per-(jj,o)
    segment sums recovered from differences at 16-element boundaries.
    """
    import numpy as np

    from concourse import dve_ops as dops
    from concourse.dve_spec import AluOp, Spec, Src0, Src1, lower, scan
    from concourse.dve_uop import DveOpSpec

    name = "SCAN_MAC_ANT"
    for op in dops.OPS:
        if op.name == name:
            return op
    spec = Spec(
        body=scan(AluOp.ADD, Src0 * Src1),
        reference=lambda in0, in1, c0, c1, c2: np.cumsum(
            np.asarray(in0, np.float32).reshape(in0.shape[0], -1)
            * np.asarray(in1, np.float32).reshape(in1.shape[0], -1),
            axis=-1,
        ).reshape(in0.shape),
    )
    shas = {}
    for ver in ("v3", "v4"):
        uops = lower(spec, ver=ver)
        shas[ver] = DveOpSpec(
            name=name, opcode=0, uops=uops, rd1_en=True
        ).sha(ver)
    op = dops.DveOp(name, spec, subdim=False, uops_sha=shas)
    dops.OPS.append(op)
    dops.CUSTOM_DVE_SPECS[name] = spec
    dops._SUB_OPCODE_FOR_NAME[name] = dops._CUSTOM_DVE_ROW_BASE + len(dops.OPS) - 1
    assert dops._SUB_OPCODE_FOR_NAME[name] < 0x20
    return op


_SCAN_MAC = _register_scan_mac()

# Problem constants (per core)
B = 32          # local batch (256 / 8 cores)
I = 1152        # in capsules
O = 10          # out capsules
D = 16          # out dim
K = 8           # in dim
JJ = 72         # i groups of 16
G = 16          # group size
OD = O * D      # 160
BB = 8          # batch block (psum/output partition packing)
NBLK = B // BB  # 4
N_ITERS = 3
AC = 24         # jj per agreement-scan chunk (3 chunks per pass)


def _ap(base, free_dims, extra_offset=0):
    """AP with the base's partition dim and explicit free [step, count] dims."""
    return bass.AP(
        tensor=base.tensor,
        offset=base.offset + extra_offset,
        ap=[list(base.ap[0])] + [list(d) for d in free_dims],
    )


def _squash(nc, pool, s_sb, p, v_out):
    """squash over d (16) per o segment. s_sb: [p, 160] f32 sbuf -> v_out."""
    sq = pool.tile([p, OD], F32, tag="sq")
    nc.scalar.square(sq, s_sb)
    nsq = pool.tile([p, O], F32, tag="nsq")
    nc.vector.reduce_sum(
        out=nsq, in_=sq[:].rearrange("p (o d) -> p o d", d=D),
        axis=mybir.AxisListType.X,
    )
    # sqrt(x) = exp(0.5*ln(x)) — keeps ACT on one table set (ln/exp)
    rt = pool.tile([p, O], F32, tag="rt")
    nc.scalar.activation(rt, nsq, mybir.ActivationFunctionType.Ln)
    nc.scalar.activation(rt, rt, mybir.ActivationFunctionType.Exp, scale=0.5)
    nc.vector.tensor_scalar_add(rt, rt, 1e-8)     # + eps
    op1 = pool.tile([p, O], F32, tag="op1")
    nc.vector.tensor_scalar_add(op1, nsq, 1.0)    # 1 + |s|^2
    nc.vector.tensor_mul(op1, op1, rt)            # (1+n)(sqrt+eps)
    rec = pool.tile([p, O], F32, tag="rec")
    nc.vector.reciprocal(rec, op1)
    nc.vector.tensor_mul(rec, rec, nsq)           # n/((1+n)(sqrt+eps))
    nc.vector.tensor_mul(
        v_out[:].rearrange("p (o d) -> p o d", d=D),
        s_sb[:].rearrange("p (o d) -> p o d", d=D),
        _ap(rec[:], [[1, O], [0, D]]),
    )
    return v_out


def _pin_act_table():
    """Make every ACT function we use resolve to the one set containing all
    of them (natural_log_exp_and_others), so bacc hoists a single
    InstLoadActFuncSet instead of thrashing Exp<->Ln sets (~1.3us/load)."""
    from concourse.bacc import get_activation_tables

    tabs = get_activation_tables("gen3")
    keep = "natural_log_exp_and_others"
    if keep not in tabs:
        return
    ours = {
        mybir.ActivationFunctionType.Exp,
        mybir.ActivationFunctionType.Ln,
        mybir.ActivationFunctionType.Square,
        mybir.ActivationFunctionType.Copy,
        mybir.ActivationFunctionType.Identity,
    }
    if not ours <= tabs[keep]:
        return
    for name, s in tabs.items():
        if name != keep:
            s -= ours


def build_program():
    _pin_act_table()
    nc = bacc.Bacc("TRN2")
    wr_d = nc.dram_tensor("wr", [128, JJ * OD], BF16, kind="ExternalInput")
    ut_d = nc.dram_tensor("ut", [128, JJ * B], BF16, kind="ExternalInput")
    # block-diag u, host-packed contiguous per (blk, half): [4, 2, 128, 4*1152]
    bdu_d = nc.dram_tensor(
        "bdu", [NBLK * 2 * 128, 4 * 9 * BB * G], BF16, kind="ExternalInput"
    )
    mb_d = nc.dram_tensor("maskb", [128, BB * O], BF16, kind="ExternalInput")
    md_d = nc.dram_tensor("maskd", [128, OD], F32, kind="ExternalInput")
    # batch-broadcast masks: vrep matmul constants
    mbg_d = nc.dram_tensor("maskbg", [O * BB, 128], BF16, kind="ExternalInput")
    mB_d = nc.dram_tensor("maskB", [B, NBLK * 128], BF16, kind="ExternalInput")
    out_d = nc.dram_tensor("v_out", [B, OD], F32, kind="ExternalOutput")

    with tile.TileContext(nc) as tc:
        with (
            tc.tile_pool(name="persist", bufs=1) as persist,
            tc.tile_pool(name="uhat", bufs=3) as uhat_pool,
            tc.tile_pool(name="bdu", bufs=4) as bdu_pool,
            tc.tile_pool(name="ascr", bufs=2) as ascr_pool,
            tc.tile_pool(name="cbd", bufs=2) as cbd_pool,
            tc.tile_pool(name="blog", bufs=2) as blog_pool,
            tc.tile_pool(name="cbuf", bufs=2) as cbuf_pool,
            tc.tile_pool(name="small", bufs=2) as small,
            tc.tile_pool(name="pb", bufs=2, space="PSUM") as pb_pool,
            tc.tile_pool(name="ps", bufs=2, space="PSUM") as ps_pool,
            tc.tile_pool(name="ps0", bufs=1, space="PSUM") as ps0_pool,
            tc.tile_pool(name="psv", bufs=1, space="PSUM") as psv_pool,
        ):
            # ---- resident loads, explicitly spread over the SP/Act HWDGE
            # queues + Pool SWDGE so cold-start transfers run in parallel.
            # Queue order == emission order per engine:
            #   Act: wr0-3, ut, masks   SP: bdu00, wr4-7, bdu01
            #   Pool: bdu1 h0/h1
            wr = persist.tile([128, JJ, OD], BF16)

            def load_wr(ch, eng):
                eng.dma_start(
                    out=wr[:, ch * 9 : (ch + 1) * 9, :],
                    in_=wr_d[:, ch * 9 * OD : (ch + 1) * 9 * OD].rearrange(
                        "p (a b) -> p a b", b=OD
                    ),
                )

            bdu_tiles = {}

            def load_bdu(blk, h, eng):
                bdu = bdu_pool.tile(
                    [128, 4, 9, BB, G], BF16, tag="bdu", name="bdu"
                )
                bdu_tiles[(blk, h)] = bdu
                eng.dma_start(
                    out=bdu,
                    in_=bdu_d[
                        (blk * 2 + h) * 128 : (blk * 2 + h + 1) * 128, :
                    ].rearrange("p (c a b g) -> p c a b g", a=9, b=BB, g=G),
                )

            # DMA transfers are globally serialized at full aggregate
            # bandwidth, so emission order == transfer order. Dependency
            # order: ut+early masks, wr first half (s0 start + build0-h0),
            # bdu00, wr rest, bdu01, late masks, bdu1.
            ut = persist.tile([128, JJ, B], BF16)
            nc.sync.dma_start(
                out=ut, in_=ut_d[:].rearrange("p (a b) -> p a b", b=B)
            )
            maskB = persist.tile([B, NBLK * 128], BF16, tag="maskB")
            nc.sync.dma_start(out=maskB, in_=mB_d[:])
            maskb = persist.tile([128, BB * O], BF16)
            nc.sync.dma_start(out=maskb, in_=mb_d[:])
            for ch in range(8):
                load_wr(ch, nc.sync)
            load_bdu(0, 0, nc.sync)
            load_bdu(0, 1, nc.sync)
            maskd = persist.tile([128, OD], F32)
            nc.sync.dma_start(out=maskd, in_=md_d[:])
            maskbg = persist.tile([O * BB, 128], BF16, tag="maskbg")
            nc.sync.dma_start(out=maskbg, in_=mbg_d[:])
            load_bdu(1, 0, nc.sync)
            load_bdu(1, 1, nc.sync)

            # PE p-state warm-up: keep the PE continuously busy from t~0 so
            # the ramp (0.65->2.4GHz after 3us busy) completes before s0's
            # real matmuls; outputs land in s0's psum bank and are discarded
            # by s0's start=True reset.
            warm = persist.tile([128, 256], BF16, tag="warm")
            nc.gpsimd.memset(warm, 0.0)
            warm_ps = ps0_pool.tile([B, OD], F32, tag="s0ps", name="s0ps")
            for i in range(18):
                nc.tensor.matmul(
                    warm_ps, lhsT=warm[:, :B], rhs=warm[:, :OD],
                    start=True, stop=True,
                )

            # ---- pipelined per-8-batch blocks ----
            u_hats, blogs, vcurs = {}, {}, {}
            scr_init = [0]

            def emit_build_half(blk, h):
                u_hat = u_hats[blk]
                bdu = bdu_tiles.pop((blk, h), None)
                if bdu is None:
                    bdu = bdu_pool.tile(
                        [128, 4, 9, BB, G], BF16, tag="bdu", name="bdu"
                    )
                    nc.sync.dma_start(
                        out=bdu,
                        in_=bdu_d[
                            (blk * 2 + h) * 128 : (blk * 2 + h + 1) * 128, :
                        ].rearrange(
                            "p (c a b g) -> p c a b g", a=9, b=BB, g=G
                        ),
                    )
                # 6-jj groups: two psum banks per group ([128, 1024] f32,
                # matmul outputs at in-bank offsets 0/160/320), one strided
                # psum->SBUF bf16 copy per group (split ACT/Pool 2:1)
                for grp in range(6):
                    ps = pb_pool.tile([128, 1024], F32, tag="ps")
                    jj0 = h * 36 + grp * 6
                    for jx in range(6):
                        jj = jj0 + jx
                        off = (jx // 3) * 512 + (jx % 3) * OD
                        nc.tensor.matmul(
                            ps[:, off : off + OD],
                            lhsT=bdu[:, jj % 36 // 9, jj % 9, :, :],
                            rhs=wr[:, jj, :], start=True, stop=True,
                        )
                    src = _ap(ps[:], [[512, 2], [OD, 3], [1, OD]])
                    nc.scalar.copy(u_hat[:, jj0 : jj0 + 6, :], src)

            def emit_build(blk):
                u_hats[blk] = uhat_pool.tile(
                    [128, JJ, OD], BF16, tag="u_hat", name="u_hat"
                )
                for h in range(2):
                    emit_build_half(blk, h)

            vreps = {}

            def emit_vrep(blk, t):
                """vrep[(b,g), od] = v[b, od] via PE broadcast matmul; the
                psum->SBUF copy runs on DVE (keeps the ACT copy queue free)."""
                vr_ps = psv_pool.tile([128, OD], F32, tag="vrps")
                if t == 1:
                    blogs[blk] = blog_pool.tile(
                        [128, JJ, O], F32, tag="blog", name="blog"
                    )
                    nc.tensor.matmul(
                        vr_ps,
                        lhsT=maskB[:, blk * 128 : (blk + 1) * 128],
                        rhs=v016, start=True, stop=True,
                    )
                else:
                    vprev = vcurs[blk]
                    # vdiag[(b,o), (o',d)] = vcur[(b,o), d] iff o'==o
                    vdiag = small.tile([O * BB, OD], BF16, tag="vdiag")
                    nc.vector.tensor_mul(
                        vdiag[:].rearrange("p (o d) -> p o d", d=D),
                        maskd[: O * BB, :].rearrange("p (o d) -> p o d", d=D),
                        _ap(vprev[:], [[0, O], [1, D]]),
                    )
                    nc.tensor.matmul(
                        vr_ps, lhsT=maskbg, rhs=vdiag, start=True, stop=True
                    )
                vrep = small.tile([128, OD], BF16, tag="vrep")
                nc.vector.tensor_copy(vrep, vr_ps)
                vreps[(blk, t)] = vrep

            def emit_agr(blk, t):
                """Phase A: scan-MAC agreement, logits, softmax -> cb."""
                u_hat = u_hats[blk]
                blog = blogs[blk]
                vrep = vreps.pop((blk, t))
                # -- agreement: chunked fused scan-MAC --
                # S = cumsum(u_hat * v); per-(jj,o) sums from boundary
                # differences S[16n+15] - S[16n-1]. The scan writes at
                # offset D into the scratch tile whose first D slots are
                # permanently zero, so segment 0's "low" reads the zero
                # slot and the whole diff is one subtract.
                NSEG = AC * O
                for h in range(JJ // AC):
                    scr = ascr_pool.tile([128, D + AC * OD], F32, tag="scr")
                    if scr_init[0] < 2:
                        scr_init[0] += 1
                        nc.gpsimd.memset(scr[:, :D], 0.0)
                    sv = scr[:]
                    nc.vector._custom_dve(
                        _SCAN_MAC,
                        out=bass.AP(
                            tensor=sv.tensor, offset=sv.offset + D,
                            ap=[list(sv.ap[0]), [1, AC * OD]],
                        ),
                        in0=u_hat[:, h * AC : (h + 1) * AC, :],
                        in1=_ap(vrep[:], [[0, AC], [1, OD]]),
                    )
                    s_hi = bass.AP(
                        tensor=sv.tensor, offset=sv.offset + 2 * D - 1,
                        ap=[list(sv.ap[0]), [D, NSEG]],
                    )
                    s_lo = bass.AP(
                        tensor=sv.tensor, offset=sv.offset + D - 1,
                        ap=[list(sv.ap[0]), [D, NSEG]],
                    )
                    bl = blog[:, h * AC : (h + 1) * AC, :]
                    bl_flat = bl.rearrange("p a o -> p (a o)")
                    if t == 1:
                        nc.gpsimd.tensor_sub(bl_flat, s_hi, s_lo)
                    else:
                        dif = small.tile([128, NSEG], F32, tag="dif")
                        nc.gpsimd.tensor_sub(dif, s_hi, s_lo)
                        nc.gpsimd.tensor_add(bl_flat, bl_flat, dif)

                # -- c = softmax(blog) over o --
                # logits are bounded (||v||<1 => |logit| <~ 16),
                # so exp without max-subtraction is fp32-safe
                cb = cbuf_pool.tile([128, JJ, O], BF16, tag="cb")
                nc.scalar.activation(cb, blog, mybir.ActivationFunctionType.Exp)
                ssum = small.tile([128, JJ], F32, tag="ssum")
                nc.vector.reduce_sum(
                    out=ssum, in_=cb, axis=mybir.AxisListType.X
                )
                rec = small.tile([128, JJ], F32, tag="srec")
                nc.vector.reciprocal(rec, ssum)
                nc.gpsimd.tensor_mul(cb, cb, _ap(rec[:], [[1, JJ], [0, O]]))
                return cb

            s_lnch = {}

            def emit_s_launch(blk, t, cb):
                """Phase B1: block-diag c + s matmuls (no DVE consumer yet,
                so the next pass's scans can run during the PE burst)."""
                u_hat = u_hats[blk]
                # block-diag c, 2x-mode bf16 mults in two halves so the
                # s matmuls can start after the first half:
                # cbd[p,(jj,b',o)] = cb[p,(jj,o)] * maskb[p,(b',o)]
                cbd = cbd_pool.tile([128, JJ, BB, O], BF16, tag="cbd")
                s_ps = ps_pool.tile([BB * O, OD], F32, tag="s_ps")
                # p-state preheat: ramp the PE in the exp->cbd window (the
                # cb dependency delays these to just before the real burst);
                # results are discarded by the start=True reset below
                for i in range(10):
                    nc.tensor.matmul(
                        s_ps,
                        lhsT=_ap(cb[:], [[1, BB * O]]),
                        rhs=_ap(cb[:], [[1, OD]]),
                        start=True, stop=True,
                    )
                for hh in range(2):
                    j0 = hh * (JJ // 2)
                    nc.vector.tensor_mul(
                        cbd[:, j0 : j0 + JJ // 2, :, :],
                        _ap(cb[:], [[O, JJ // 2], [0, BB], [1, O]],
                            extra_offset=j0 * O),
                        _ap(maskb[:], [[0, JJ // 2], [O, BB], [1, O]]),
                    )
                    for jj in range(j0, j0 + JJ // 2):
                        nc.tensor.matmul(
                            s_ps, lhsT=cbd[:, jj, :, :], rhs=u_hat[:, jj, :],
                            start=(jj == 0), stop=(jj == JJ - 1),
                        )
                s_lnch[(blk, t)] = s_ps

            def emit_s_finish(blk, t):
                """Phase B2: diag extract + squash (waits on the s matmuls)."""
                s_ps = s_lnch.pop((blk, t))
                # -- diag extract: s80[(b,o), d] = s_ps[(b,o), o*16+d]
                #    via constant diag mask + reduce over o' --
                sdm = small.tile([O * BB, OD], F32, tag="sdm")
                nc.vector.tensor_mul(sdm, s_ps, maskd[: O * BB, :])
                s80 = small.tile([O * BB, D], F32, tag="s80")
                nc.vector.reduce_sum(
                    out=s80,
                    in_=sdm[:].rearrange("p (o d) -> p d o", d=D),
                    axis=mybir.AxisListType.X,
                )
                # squash on [(b,o), d] with per-partition scalars
                nsq = small.tile([O * BB, 1], F32, tag="nsq80")
                sq = small.tile([O * BB, D], F32, tag="sq80")
                nc.vector.tensor_mul(sq, s80, s80)
                nc.vector.reduce_sum(out=nsq, in_=sq, axis=mybir.AxisListType.X)
                # squash factor ~= sqrt(nsq)/(1+nsq)  (eps negligible);
                # sqrt via exp(0.5*ln) to stay on one ACT table set
                rt = small.tile([O * BB, 1], F32, tag="rt80")
                nc.scalar.activation(rt, nsq, mybir.ActivationFunctionType.Ln)
                nc.scalar.activation(
                    rt, rt, mybir.ActivationFunctionType.Exp, scale=0.5
                )
                op1 = small.tile([O * BB, 1], F32, tag="op180")
                nc.vector.tensor_scalar_add(op1, nsq, 1.0)
                rec80 = small.tile([O * BB, 1], F32, tag="rec80")
                nc.vector.reciprocal(rec80, op1)
                nc.vector.tensor_mul(rec80, rec80, rt)
                vcur = small.tile([O * BB, D], F32, tag="vcur")
                nc.vector.tensor_scalar_mul(vcur, s80, rec80)
                vcurs[blk] = vcur
                if t == N_ITERS - 1:
                    # v_out[blk*8+b, o*16+d] = vcur[b*10+o, d] (same order)
                    nc.sync.dma_start(
                        out=out_d[blk * BB : (blk + 1) * BB, :], in_=vcur
                    )

            cbs = {}

            def V(blk, t):
                emit_vrep(blk, t)

            def A(blk, t):
                cbs[(blk, t)] = emit_agr(blk, t)

            def L(blk, t):
                emit_s_launch(blk, t, cbs.pop((blk, t)))

            def F(blk, t):
                emit_s_finish(blk, t)

            # software pipeline: s0's accumulation is split around
            # build0-h0 so the PE stream follows DMA arrival order (ut+wr
            # first half -> bdu00 -> wr second half -> bdu01); the s0 tail
            # chain runs on Pool/DVE so it doesn't head-of-line block the
            # ACT copy queue.
            s0_ps = warm_ps
            u_hats[0] = uhat_pool.tile(
                [128, JJ, OD], BF16, tag="u_hat", name="u_hat"
            )
            for jj in range(JJ):
                nc.tensor.matmul(
                    s0_ps, lhsT=ut[:, jj, :], rhs=wr[:, jj, :],
                    start=(jj == 0), stop=(jj == JJ - 1),
                )
            s0_sb = small.tile([B, OD], F32, tag="s0")
            nc.vector.tensor_scalar_mul(s0_sb, s0_ps, 0.1)
            v0 = persist.tile([B, OD], F32, tag="v0")
            _squash(nc, small, s0_sb, B, v0)  # [32, 160]
            v016 = persist.tile([B, OD], BF16, tag="v016")
            nc.vector.tensor_copy(v016, v0)
            emit_build_half(0, 0)
            emit_build_half(0, 1)
            V(0, 1)
            emit_build(1)
            # prefetch the remaining block-diag streams: transfers run on
            # the serialized DMA device well before builds 2/3 need them
            load_bdu(2, 0, nc.sync)
            load_bdu(2, 1, nc.sync)
            load_bdu(3, 0, nc.sync)
            load_bdu(3, 1, nc.sync)
            A(0, 1)
            V(1, 1)
            A(1, 1)
            L(0, 1)
            L(1, 1)
            F(0, 1)
            V(0, 2)
            A(0, 2)
            F(1, 1)
            L(0, 2)
            emit_build(2)
            V(1, 2)
            A(1, 2)
            F(0, 2)
            L(1, 2)
            emit_build(3)
            V(2, 1)
            A(2, 1)
            F(1, 2)
            L(2, 1)
            V(3, 1)
            A(3, 1)
            F(2, 1)
            L(3, 1)
            V(2, 2)
            A(2, 2)
            F(3, 1)
            L(2, 2)
            V(3, 2)
            A(3, 2)
            L(3, 2)
            F(2, 2)
            F(3, 2)
    nc.compile()
    return nc


# ---------------- host side ----------------

_NC_CACHE = None


def _get_nc():
    global _NC_CACHE
    if _NC_CACHE is None:
        _NC_CACHE = build_program()
    return _NC_CACHE


def _bf16(a):
    import ml_dtypes

    return np.ascontiguousarray(a).astype(ml_dtypes.bfloat16)


def _pack_wr(W):
    # Wr[g*8+k, jj*160 + o*16 + d] = W[jj*16+g, o, d, k]
    return _bf16(
        W.reshape(JJ, G, O, D, K).transpose(1, 4, 0, 2, 3).reshape(128, JJ * OD)
    )


def _pack_ut(u_loc):
    # uT[g*8+k, jj*B + b] = u_loc[b, jj*16+g, k]
    return _bf16(
        u_loc.reshape(B, JJ, G, K).transpose(2, 3, 1, 0).reshape(128, JJ * B)
    )


def _masks():
    p = np.arange(128)
    mb = (np.arange(BB)[None, :] == (p // G)[:, None]).astype(np.float32)
    mb = np.repeat(mb, O, axis=1)  # [128, 80] over (b', o)
    # maskd[(b,o) p<80, o'*16+d] = (o' == o); rows >=80 zero
    md = np.zeros((128, OD), dtype=np.float32)
    po = np.arange(O * BB) % O
    for od in range(OD):
        md[: O * BB, od] = (od // D == po).astype(np.float32)
    # maskbg[(b,o), (b',g)] = (b' == b): vrep broadcast from vcur-diag
    pb = np.arange(O * BB) // O
    mbg = (pb[:, None] == (np.arange(128) // G)[None, :]).astype(np.float32)
    # maskB[b, blk*128 + (b',g)] = (b == blk*8 + b'): vrep broadcast from v0
    bidx = np.arange(B)
    cols = (np.arange(NBLK)[:, None] * BB + (np.arange(128) // G)[None, :])
    mB = (bidx[:, None] == cols.reshape(1, -1)).astype(np.float32)
    return _bf16(mb), md, _bf16(mbg), _bf16(mB)


def _pack_bdu(u_loc):
    # bdu[(blk,h)*128 + g*8+k, (ch4, j, b, g')] =
    #   u_loc[blk*8+b, ((h*4+ch4)*9+j)*16+g', k], nonzero only when g' == g;
    #   contiguous per (blk, half) slice (one DMA each).
    u4 = u_loc.reshape(NBLK, BB, JJ // 9, 9, G, K)  # (blk, b, ch, j, g, k)
    out = np.zeros((NBLK, 8, G, K, 9, BB, G), dtype=np.float32)
    for g in range(G):
        # (blk, ch, k, j, b)
        out[:, :, g, :, :, :, g] = u4[:, :, :, :, g, :].transpose(0, 2, 4, 3, 1)
    # (blk, ch8, g, k, j, b, g') -> (blk, h2, g, k, ch4, j, b, g')
    out = out.reshape(NBLK, 2, 4, G, K, 9, BB, G).transpose(
        0, 1, 3, 4, 2, 5, 6, 7
    )
    return _bf16(out.reshape(NBLK * 2 * 128, 4 * 9 * BB * G))


LAST_RESULTS = None


def kernel(u, W):
    from concourse.bass_utils import run_bass_kernel_spmd

    global LAST_RESULTS
    u = np.asarray(u, dtype=np.float32)
    W = np.asarray(W, dtype=np.float32)
    nc = _get_nc()
    wr = _pack_wr(W)
    mb, md, mbg, mB = _masks()
    in_maps = []
    for c in range(8):
        u_loc = u[c * B : (c + 1) * B]
        in_maps.append(
            {
                "wr": wr,
                "ut": _pack_ut(u_loc),
                "bdu": _pack_bdu(u_loc),
                "maskb": mb,
                "maskd": md,
                "maskbg": mbg,
                "maskB": mB,
            }
        )
    trace = bool(int(os.environ.get("KBENCH_TRACE", "0")))
    try:
        res = run_bass_kernel_spmd(
            nc, in_maps, core_ids=list(range(8)), trace=trace
        )
    except ModuleNotFoundError:
        # axon NTFF hook unavailable in this container; run without trace
        res = run_bass_kernel_spmd(nc, in_maps, core_ids=list(range(8)))
    LAST_RESULTS = res
    outs = [r["v_out"].reshape(B, O, D) for r in res.results]
    return np.concatenate(outs, axis=0).astype(np.float32)
